# revision 11
# baseline (speedup 1.0000x reference)
"""Linear attention (B=4, S=4096, D=1024, H=16) on 8 TRN2 NeuronCores.

Sharding: core = (batch, head-half): each core handles one batch's 8 heads.
 - x is host-transposed to xT [D, S] per batch so both operand orientations
   of every matmul come out of the tensor engine with no on-device transpose.
 - Wqkv column-sharded per head-half; Wo row-sharded; host sums the two
   partial y's per batch (row-parallel unshard).

Per-core dataflow (S=4096 in 8 blocks of 512 tokens):
  phase A: qkv projection (fp32r):
      QT [512f, S] feature-major  (lhsT=Wq, rhs=xT)   -> elu+1 -> bf16 QT
      K,V [S, 512f] token-major   (lhsT=xT, rhs=Wkv)  -> elu+1(K), copy(V) bf16
  phase B: per head-pair KV accumulation in PSUM over all tokens (bf16):
      kv_psum[p][:,0:128]  += K_pair^T @ V_pair
      kv_psum[p][:,128]    += K_pair^T @ ones      (= K_sum^T)
  phase C: per head h: psum[0:65,s] = [KV_h | Ksum_h]^T-fold @ QT_h
      row 64 = normalizer; rcp = 1/(norm+eps) (ACT); rcpb = ones x rcp (PE);
      outT = psum[0:64] * rcpb (DVE, bf16)   [feature-major out]
  phase D: y[s,:] (+)= outT^T @ Wo  (bf16 matmul, fp32 out)
"""

import numpy as np

import concourse.bacc as bacc
import concourse.mybir as mybir
import concourse.tile as tile
from concourse.bass_utils import run_bass_kernel_spmd

F32 = mybir.dt.float32
F32R = mybir.dt.float32r
BF16 = mybir.dt.bfloat16

P = 128
B, S, D = 4, 4096, 1024
H = 16
HD = 64
EPS = 1e-6

FSH = 512            # features per core for each of Q, K, V (8 heads)
KSUB = D // P        # 8 contraction subtiles
SBLK = 512           # tokens per block
NBLK = S // SBLK     # 8 blocks
TSUB = SBLK // P     # 4 token subtiles per block
NPAIR = 4            # head pairs per core
NHEAD = 8            # heads per core

_NC_CACHE = None


def build():
    nc = bacc.Bacc(target_bir_lowering=False)
    xT = nc.dram_tensor("xT", [D, S], F32, kind="ExternalInput")
    wqkv = nc.dram_tensor("wqkv", [D, 3 * FSH], F32, kind="ExternalInput")
    wo = nc.dram_tensor("wo", [FSH, D], BF16, kind="ExternalInput")
    y = nc.dram_tensor("y", [S, D], F32, kind="ExternalOutput")

    xT_r = xT.rearrange("(ko p) s -> p ko s", p=P)        # [128, 8, 4096]
    wqkv_r = wqkv.rearrange("(ko p) f -> p ko f", p=P)    # [128, 8, 1536]
    wo_r = wo.rearrange("(fo p) n -> p fo n", p=P)        # [128, 4, 1024]
    y_r = y.rearrange("(j t p) n -> j p t n", t=TSUB, p=P)  # [8, 128, 4, 1024]

    with tile.TileContext(nc) as tc:
        import contextlib

        with contextlib.ExitStack() as ctx:
            const = ctx.enter_context(tc.tile_pool(name="const", bufs=1))
            wpool = ctx.enter_context(tc.tile_pool(name="wpool", bufs=1))
            qtpool = ctx.enter_context(tc.tile_pool(name="qtpool", bufs=1))

            # persistent SBUF
            wqkv_sb = wpool.tile([P, KSUB, 3 * FSH], F32R)
            nc.sync.dma_start(out=wqkv_sb, in_=wqkv_r.bitcast(F32R))
            wo_sb = wpool.tile([P, FSH // P, D], BF16)
            nc.sync.dma_start(out=wo_sb, in_=wo_r)
            qt_sb = qtpool.tile([P, FSH // P, S], BF16)   # feature-major Q
            lhsT_sb = qtpool.tile([P, NHEAD, HD + 1], BF16)

            ones_f32 = const.tile([P, HD], F32)
            nc.vector.memset(ones_f32, 1.0)
            ones_fr = const.tile([P, HD], F32R)
            with nc.allow_low_precision(reason="fp32r ones constant"):
                nc.vector.tensor_copy(out=ones_fr, in_=ones_f32)

            # ---------------- phase A + B ----------------
            with (
                tc.tile_pool(name="kvps", bufs=1, space="PSUM") as kvps_pool,
                tc.tile_pool(name="xin", bufs=2) as xpool,
                tc.tile_pool(name="stage", bufs=2) as stpool,
                tc.tile_pool(name="paps", bufs=3, space="PSUM") as pa_ps,
                tc.tile_pool(name="etmp", bufs=3) as etpool,
            ):
                kvps = [
                    kvps_pool.tile([P, P + 1], F32, tag=f"kv{p}", name=f"kv{p}")
                    for p in range(NPAIR)
                ]

                for j in range(NBLK):
                    xt = xpool.tile([P, KSUB, SBLK], F32R, tag="xt")
                    nc.sync.dma_start(
                        out=xt,
                        in_=xT_r[:, :, j * SBLK : (j + 1) * SBLK].bitcast(F32R),
                    )

                    # QT: 4 feature blocks of 128
                    for f in range(FSH // P):
                        ps = pa_ps.tile([P, SBLK], F32, tag="pa")
                        for k in range(KSUB):
                            nc.tensor.matmul(
                                ps,
                                wqkv_sb[:, k, f * P : (f + 1) * P],
                                xt[:, k, :],
                                start=(k == 0),
                                stop=(k == KSUB - 1),
                            )
                        # elu(x)+1 = min(exp(x),1) + relu(x)
                        e = etpool.tile([P, SBLK], F32, tag="e")
                        nc.scalar.activation(
                            out=e, in_=ps, func=mybir.ActivationFunctionType.Exp
                        )
                        r = etpool.tile([P, SBLK], F32, tag="r")
                        nc.vector.tensor_scalar_max(r, ps, 0.0)
                        nc.vector.scalar_tensor_tensor(
                            out=qt_sb[:, f, j * SBLK : (j + 1) * SBLK],
                            in0=e,
                            scalar=1.0,
                            in1=r,
                            op0=mybir.AluOpType.min,
                            op1=mybir.AluOpType.add,
                        )

                    # K, V token-major per 128-token subtile.
                    # vst carries a ones column per head-pair slot so one
                    # matmul accumulates both KV and K_sum^T.
                    kst = stpool.tile([P, TSUB, FSH], BF16, tag="kst")
                    vst = stpool.tile([P, TSUB, NPAIR, P + 1], BF16, tag="vst")
                    nc.vector.memset(vst[:, :, :, P : P + 1], 1.0)
                    for t in range(TSUB):
                        psk = pa_ps.tile([P, FSH], F32, tag="pa")
                        for k in range(KSUB):
                            nc.tensor.matmul(
                                psk,
                                xt[:, k, t * P : (t + 1) * P],
                                wqkv_sb[:, k, FSH : 2 * FSH],
                                start=(k == 0),
                                stop=(k == KSUB - 1),
                            )
                        e = etpool.tile([P, SBLK], F32, tag="e")
                        nc.scalar.activation(
                            out=e, in_=psk, func=mybir.ActivationFunctionType.Exp
                        )
                        r = etpool.tile([P, SBLK], F32, tag="r")
                        nc.vector.tensor_scalar_max(r, psk, 0.0)
                        nc.vector.scalar_tensor_tensor(
                            out=kst[:, t, :],
                            in0=e,
                            scalar=1.0,
                            in1=r,
                            op0=mybir.AluOpType.min,
                            op1=mybir.AluOpType.add,
                        )

                        psv = pa_ps.tile([P, FSH], F32, tag="pa")
                        for k in range(KSUB):
                            nc.tensor.matmul(
                                psv,
                                xt[:, k, t * P : (t + 1) * P],
                                wqkv_sb[:, k, 2 * FSH : 3 * FSH],
                                start=(k == 0),
                                stop=(k == KSUB - 1),
                            )
                        nc.scalar.copy(out=vst[:, t, :, 0:P], in_=psv)

                    # phase B: accumulate [KV | K_sum^T] into persistent psums
                    first = j == 0
                    last = j == NBLK - 1
                    for t in range(TSUB):
                        for p_ in range(NPAIR):
                            nc.tensor.matmul(
                                kvps[p_],
                                kst[:, t, p_ * P : (p_ + 1) * P],
                                vst[:, t, p_, :],
                                start=(first and t == 0),
                                stop=(last and t == TSUB - 1),
                            )

                # assemble per-head lhsT = [KV_h | Ksum_h] (bf16)
                for h in range(NHEAD):
                    p_ = h // 2
                    base = (h % 2) * HD
                    nc.vector.tensor_copy(
                        out=lhsT_sb[base : base + HD, h, 0:HD],
                        in_=kvps[p_][base : base + HD, base : base + HD],
                    )
                    nc.vector.tensor_copy(
                        out=lhsT_sb[base : base + HD, h, HD : HD + 1],
                        in_=kvps[p_][base : base + HD, P : P + 1],
                    )

            # ---------------- phase C + D ----------------
            with (
                tc.tile_pool(name="pcps", bufs=2, space="PSUM") as pc_ps,
                tc.tile_pool(name="prps", bufs=2, space="PSUM") as pr_ps,
                tc.tile_pool(name="pyps", bufs=2, space="PSUM") as py_ps,
                tc.tile_pool(name="cd", bufs=3) as cdpool,
                tc.tile_pool(name="yout", bufs=2) as ypool,
            ):
                for j in range(NBLK):
                    outt = cdpool.tile([P, FSH // P, SBLK], BF16, tag="outt")
                    for h in range(NHEAD):
                        base = (h % 2) * HD
                        psc = pc_ps.tile([P, SBLK], F32, tag="pc")
                        nc.tensor.matmul(
                            psc[0 : HD + 1, :],
                            lhsT_sb[base : base + HD, h, :],
                            qt_sb[base : base + HD, h // 2, j * SBLK : (j + 1) * SBLK],
                            start=True,
                            stop=True,
                        )
                        # rcp = 1/(norm + eps) on the normalizer row
                        nrm = cdpool.tile([P, SBLK], F32, tag="nrm")
                        nc.vector.tensor_scalar_add(
                            nrm[HD : HD + 1, :], psc[HD : HD + 1, :], EPS
                        )
                        rcp = cdpool.tile([P, SBLK], F32R, tag="rcp")
                        with nc.allow_low_precision(
                            reason="fp32r is 32-bit; needed as fp32r matmul operand"
                        ):
                            nc.vector.reciprocal(
                                out=rcp[HD : HD + 1, :], in_=nrm[HD : HD + 1, :]
                            )
                        # broadcast rcp across 64 partitions via PE outer product
                        psr = pr_ps.tile([P, SBLK], F32, tag="pr")
                        nc.tensor.matmul(
                            psr[0:HD, :],
                            ones_fr[HD : HD + 1, 0:HD],
                            rcp[HD : HD + 1, :],
                            start=True,
                            stop=True,
                        )
                        rcpb = cdpool.tile([P, SBLK], F32, tag="rcpb")
                        nc.scalar.copy(out=rcpb[0:HD, :], in_=psr[0:HD, :])
                        # outT_h = psum_rows * rcpb (cast bf16)
                        nc.vector.tensor_tensor(
                            out=outt[base : base + HD, h // 2, :],
                            in0=psc[0:HD, :],
                            in1=rcpb[0:HD, :],
                            op=mybir.AluOpType.mult,
                        )

                    # phase D for this block
                    ysb = ypool.tile([P, TSUB, D], F32, tag="ysb")
                    for t in range(TSUB):
                        for nb in range(D // 512):
                            psy = py_ps.tile([P, 512], F32, tag="py")
                            for fs in range(FSH // P):
                                nc.tensor.matmul(
                                    psy,
                                    outt[:, fs, t * P : (t + 1) * P],
                                    wo_sb[:, fs, nb * 512 : (nb + 1) * 512],
                                    start=(fs == 0),
                                    stop=(fs == FSH // P - 1),
                                )
                            nc.scalar.copy(
                                out=ysb[:, t, nb * 512 : (nb + 1) * 512], in_=psy
                            )
                    nc.sync.dma_start(out=y_r[j], in_=ysb)

    nc.compile()
    return nc


def _prep_inputs(x, Wqkv, Wo):
    import ml_dtypes

    x = np.ascontiguousarray(x, dtype=np.float32)
    Wqkv = np.ascontiguousarray(Wqkv, dtype=np.float32)
    Wo = np.ascontiguousarray(Wo, dtype=np.float32)
    in_maps = []
    for b in range(B):
        xT = np.ascontiguousarray(x[b].T)  # [D, S]
        for hh in range(2):
            cols = slice(hh * FSH, (hh + 1) * FSH)
            wq = Wqkv[:, 0 * D :][:, cols]
            wk = Wqkv[:, 1 * D :][:, cols]
            wv = Wqkv[:, 2 * D :][:, cols]
            wqkv_sh = np.ascontiguousarray(np.concatenate([wq, wk, wv], axis=1))
            wo_sh = np.ascontiguousarray(Wo[hh * FSH : (hh + 1) * FSH, :]).astype(
                ml_dtypes.bfloat16
            )
            in_maps.append({"xT": xT, "wqkv": wqkv_sh, "wo": wo_sh})
    return in_maps


def kernel(x, Wqkv, Wo):
    global _NC_CACHE
    if _NC_CACHE is None:
        _NC_CACHE = build()
    nc = _NC_CACHE
    in_maps = _prep_inputs(x, Wqkv, Wo)
    res = run_bass_kernel_spmd(nc, in_maps, list(range(2 * B))).results
    y = np.empty((B, S, D), dtype=np.float32)
    for b in range(B):
        y[b] = res[2 * b]["y"] + res[2 * b + 1]["y"]
    return y


# revision 15
# speedup vs baseline: 1.0419x; 1.0419x over previous
"""Linear attention (B=4, S=4096, D=1024, H=16) on 8 TRN2 NeuronCores.

Sharding: core = (batch, head-half): each core handles one batch's 8 heads.
 - x is host-transposed to xT [D, S] per batch so both operand orientations
   of every matmul come out of the tensor engine with no on-device transpose.
 - Wqkv column-sharded per head-half; Wo row-sharded; host sums the two
   partial y's per batch (row-parallel unshard).

Per-core dataflow (S=4096 in 8 blocks of 512 tokens):
  phase A: qkv projection (fp32r):
      QT [512f, S] feature-major  (lhsT=Wq, rhs=xT)   -> elu+1 -> bf16 QT
      K,V [S, 512f] token-major   (lhsT=xT, rhs=Wkv)  -> elu+1(K), copy(V) bf16
  phase B: per head-pair KV accumulation in PSUM over all tokens (bf16):
      kv_psum[p][:,0:128]  += K_pair^T @ V_pair
      kv_psum[p][:,128]    += K_pair^T @ ones      (= K_sum^T)
  phase C: per head h: psum[0:65,s] = [KV_h | Ksum_h]^T-fold @ QT_h
      row 64 = normalizer; rcp = 1/(norm+eps) (ACT); rcpb = ones x rcp (PE);
      outT = psum[0:64] * rcpb (DVE, bf16)   [feature-major out]
  phase D: y[s,:] (+)= outT^T @ Wo  (bf16 matmul, fp32 out)
"""

import numpy as np

import concourse.bacc as bacc
import concourse.mybir as mybir
import concourse.tile as tile
from concourse.bass_utils import run_bass_kernel_spmd

F32 = mybir.dt.float32
F32R = mybir.dt.float32r
BF16 = mybir.dt.bfloat16

P = 128
B, S, D = 4, 4096, 1024
H = 16
HD = 64
EPS = 1e-6

FSH = 512            # features per core for each of Q, K, V (8 heads)
KSUB = D // P        # 8 contraction subtiles
SBLK = 512           # tokens per block
NBLK = S // SBLK     # 8 blocks
TSUB = SBLK // P     # 4 token subtiles per block
NPAIR = 4            # head pairs per core
NHEAD = 8            # heads per core

_NC_CACHE = None


def build():
    nc = bacc.Bacc(target_bir_lowering=False)
    xT = nc.dram_tensor("xT", [D, S], F32, kind="ExternalInput")
    wqkv = nc.dram_tensor("wqkv", [D, 3 * FSH], F32, kind="ExternalInput")
    wo = nc.dram_tensor("wo", [FSH, D], BF16, kind="ExternalInput")
    y = nc.dram_tensor("y", [S, D], F32, kind="ExternalOutput")

    xT_r = xT.rearrange("(ko p) s -> p ko s", p=P)        # [128, 8, 4096]
    wqkv_r = wqkv.rearrange("(ko p) f -> p ko f", p=P)    # [128, 8, 1536]
    wo_r = wo.rearrange("(fo p) n -> p fo n", p=P)        # [128, 4, 1024]
    y_r = y.rearrange("(j t p) n -> j p t n", t=TSUB, p=P)  # [8, 128, 4, 1024]

    with tile.TileContext(nc) as tc:
        import contextlib

        with contextlib.ExitStack() as ctx:
            const = ctx.enter_context(tc.tile_pool(name="const", bufs=1))
            wpool = ctx.enter_context(tc.tile_pool(name="wpool", bufs=1))
            qtpool = ctx.enter_context(tc.tile_pool(name="qtpool", bufs=1))

            # persistent SBUF
            wqkv_sb = wpool.tile([P, KSUB, 3 * FSH], F32R)
            nc.sync.dma_start(out=wqkv_sb, in_=wqkv_r.bitcast(F32R))
            wo_sb = wpool.tile([P, FSH // P, D], BF16)
            nc.sync.dma_start(out=wo_sb, in_=wo_r)
            qt_sb = qtpool.tile([P, FSH // P, S], BF16)   # feature-major Q
            lhsT_sb = qtpool.tile([P, NHEAD, HD + 1], BF16)

            eps_sb = const.tile([P, 1], F32)
            nc.vector.memset(eps_sb, EPS)
            ones_f32 = const.tile([P, HD], F32)
            nc.vector.memset(ones_f32, 1.0)
            ones_fr = const.tile([P, HD], F32R)
            with nc.allow_low_precision(reason="fp32r ones constant"):
                nc.vector.tensor_copy(out=ones_fr, in_=ones_f32)

            # ---------------- phase A + B ----------------
            with (
                tc.tile_pool(name="kvps", bufs=1, space="PSUM") as kvps_pool,
                tc.tile_pool(name="xin", bufs=2) as xpool,
                tc.tile_pool(name="stage", bufs=2) as stpool,
                tc.tile_pool(name="paps", bufs=3, space="PSUM") as pa_ps,
                tc.tile_pool(name="etmp", bufs=3) as etpool,
            ):
                kvps = [
                    kvps_pool.tile([P, P + 1], F32, tag=f"kv{p}", name=f"kv{p}")
                    for p in range(NPAIR)
                ]

                for j in range(NBLK):
                    xt = xpool.tile([P, KSUB, SBLK], F32R, tag="xt")
                    nc.sync.dma_start(
                        out=xt,
                        in_=xT_r[:, :, j * SBLK : (j + 1) * SBLK].bitcast(F32R),
                    )

                    # QT: 4 feature blocks of 128
                    for f in range(FSH // P):
                        ps = pa_ps.tile([P, SBLK], F32, tag="pa")
                        for k in range(KSUB):
                            nc.tensor.matmul(
                                ps,
                                wqkv_sb[:, k, f * P : (f + 1) * P],
                                xt[:, k, :],
                                start=(k == 0),
                                stop=(k == KSUB - 1),
                            )
                        # elu(x)+1 = min(exp(x),1) + relu(x)
                        e = etpool.tile([P, SBLK], F32, tag="e")
                        nc.scalar.activation(
                            out=e, in_=ps, func=mybir.ActivationFunctionType.Exp
                        )
                        r = etpool.tile([P, SBLK], F32, tag="r")
                        nc.vector.tensor_scalar_max(r, ps, 0.0)
                        nc.vector.scalar_tensor_tensor(
                            out=qt_sb[:, f, j * SBLK : (j + 1) * SBLK],
                            in0=e,
                            scalar=1.0,
                            in1=r,
                            op0=mybir.AluOpType.min,
                            op1=mybir.AluOpType.add,
                        )

                    # K, V token-major per 128-token subtile.
                    # vst carries a ones column per head-pair slot so one
                    # matmul accumulates both KV and K_sum^T.
                    kst = stpool.tile([P, TSUB, FSH], BF16, tag="kst")
                    vst = stpool.tile([P, TSUB, NPAIR, P + 1], BF16, tag="vst")
                    nc.vector.memset(vst[:, :, :, P : P + 1], 1.0)
                    for t in range(TSUB):
                        psk = pa_ps.tile([P, FSH], F32, tag="pa")
                        for k in range(KSUB):
                            nc.tensor.matmul(
                                psk,
                                xt[:, k, t * P : (t + 1) * P],
                                wqkv_sb[:, k, FSH : 2 * FSH],
                                start=(k == 0),
                                stop=(k == KSUB - 1),
                            )
                        e = etpool.tile([P, SBLK], F32, tag="e")
                        nc.scalar.activation(
                            out=e, in_=psk, func=mybir.ActivationFunctionType.Exp
                        )
                        r = etpool.tile([P, SBLK], F32, tag="r")
                        nc.vector.tensor_scalar_max(r, psk, 0.0)
                        nc.vector.scalar_tensor_tensor(
                            out=kst[:, t, :],
                            in0=e,
                            scalar=1.0,
                            in1=r,
                            op0=mybir.AluOpType.min,
                            op1=mybir.AluOpType.add,
                        )

                        psv = pa_ps.tile([P, FSH], F32, tag="pa")
                        for k in range(KSUB):
                            nc.tensor.matmul(
                                psv,
                                xt[:, k, t * P : (t + 1) * P],
                                wqkv_sb[:, k, 2 * FSH : 3 * FSH],
                                start=(k == 0),
                                stop=(k == KSUB - 1),
                            )
                        nc.scalar.copy(out=vst[:, t, :, 0:P], in_=psv)

                    # phase B: accumulate [KV | K_sum^T] into persistent psums
                    first = j == 0
                    last = j == NBLK - 1
                    for t in range(TSUB):
                        for p_ in range(NPAIR):
                            nc.tensor.matmul(
                                kvps[p_],
                                kst[:, t, p_ * P : (p_ + 1) * P],
                                vst[:, t, p_, :],
                                start=(first and t == 0),
                                stop=(last and t == TSUB - 1),
                            )

                # assemble per-head lhsT = [KV_h | Ksum_h] (bf16)
                for h in range(NHEAD):
                    p_ = h // 2
                    base = (h % 2) * HD
                    nc.vector.tensor_copy(
                        out=lhsT_sb[base : base + HD, h, 0:HD],
                        in_=kvps[p_][base : base + HD, base : base + HD],
                    )
                    nc.vector.tensor_copy(
                        out=lhsT_sb[base : base + HD, h, HD : HD + 1],
                        in_=kvps[p_][base : base + HD, P : P + 1],
                    )

            # ---------------- phase C + D ----------------
            with (
                tc.tile_pool(name="pcps", bufs=2, space="PSUM") as pc_ps,
                tc.tile_pool(name="prps", bufs=2, space="PSUM") as pr_ps,
                tc.tile_pool(name="pyps", bufs=2, space="PSUM") as py_ps,
                tc.tile_pool(name="cd", bufs=3) as cdpool,
                tc.tile_pool(name="yout", bufs=2) as ypool,
            ):
                for j in range(NBLK):
                    outt = cdpool.tile([P, FSH // P, SBLK], BF16, tag="outt")
                    for h in range(NHEAD):
                        base = (h % 2) * HD
                        psc = pc_ps.tile([P, SBLK], F32, tag="pc")
                        nc.tensor.matmul(
                            psc[0 : HD + 1, :],
                            lhsT_sb[base : base + HD, h, :],
                            qt_sb[base : base + HD, h // 2, j * SBLK : (j + 1) * SBLK],
                            start=True,
                            stop=True,
                        )
                        # rcp = 1/(norm+eps) = exp(-ln(norm+eps)) on ACT
                        # (DVE reciprocal is an 8-slice iterative op and this
                        # row only occupies one lane — measured 3.3us each)
                        nrm = cdpool.tile([P, SBLK], F32, tag="nrm")
                        nc.scalar.activation(
                            out=nrm[HD : HD + 1, :],
                            in_=psc[HD : HD + 1, :],
                            func=mybir.ActivationFunctionType.Ln,
                            bias=eps_sb[HD : HD + 1, :],
                        )
                        rcp = cdpool.tile([P, SBLK], F32R, tag="rcp")
                        with nc.allow_low_precision(
                            reason="fp32r is 32-bit; needed as fp32r matmul operand"
                        ):
                            nc.scalar.activation(
                                out=rcp[HD : HD + 1, :],
                                in_=nrm[HD : HD + 1, :],
                                func=mybir.ActivationFunctionType.Exp,
                                scale=-1.0,
                            )
                        # broadcast rcp across 64 partitions via PE outer product
                        psr = pr_ps.tile([P, SBLK], F32, tag="pr")
                        nc.tensor.matmul(
                            psr[0:HD, :],
                            ones_fr[HD : HD + 1, 0:HD],
                            rcp[HD : HD + 1, :],
                            start=True,
                            stop=True,
                        )
                        rcpb = cdpool.tile([P, SBLK], F32, tag="rcpb")
                        nc.scalar.copy(out=rcpb[0:HD, :], in_=psr[0:HD, :])
                        # outT_h = psum_rows * rcpb (cast bf16)
                        nc.vector.tensor_tensor(
                            out=outt[base : base + HD, h // 2, :],
                            in0=psc[0:HD, :],
                            in1=rcpb[0:HD, :],
                            op=mybir.AluOpType.mult,
                        )

                    # phase D for this block
                    ysb = ypool.tile([P, TSUB, D], F32, tag="ysb")
                    for t in range(TSUB):
                        for nb in range(D // 512):
                            psy = py_ps.tile([P, 512], F32, tag="py")
                            for fs in range(FSH // P):
                                nc.tensor.matmul(
                                    psy,
                                    outt[:, fs, t * P : (t + 1) * P],
                                    wo_sb[:, fs, nb * 512 : (nb + 1) * 512],
                                    start=(fs == 0),
                                    stop=(fs == FSH // P - 1),
                                )
                            nc.vector.tensor_copy(
                                out=ysb[:, t, nb * 512 : (nb + 1) * 512], in_=psy
                            )
                    nc.sync.dma_start(out=y_r[j], in_=ysb)

    nc.compile()
    return nc


def _prep_inputs(x, Wqkv, Wo):
    import ml_dtypes

    x = np.ascontiguousarray(x, dtype=np.float32)
    Wqkv = np.ascontiguousarray(Wqkv, dtype=np.float32)
    Wo = np.ascontiguousarray(Wo, dtype=np.float32)
    in_maps = []
    for b in range(B):
        xT = np.ascontiguousarray(x[b].T)  # [D, S]
        for hh in range(2):
            cols = slice(hh * FSH, (hh + 1) * FSH)
            wq = Wqkv[:, 0 * D :][:, cols]
            wk = Wqkv[:, 1 * D :][:, cols]
            wv = Wqkv[:, 2 * D :][:, cols]
            wqkv_sh = np.ascontiguousarray(np.concatenate([wq, wk, wv], axis=1))
            wo_sh = np.ascontiguousarray(Wo[hh * FSH : (hh + 1) * FSH, :]).astype(
                ml_dtypes.bfloat16
            )
            in_maps.append({"xT": xT, "wqkv": wqkv_sh, "wo": wo_sh})
    return in_maps


def kernel(x, Wqkv, Wo):
    global _NC_CACHE
    if _NC_CACHE is None:
        _NC_CACHE = build()
    nc = _NC_CACHE
    in_maps = _prep_inputs(x, Wqkv, Wo)
    res = run_bass_kernel_spmd(nc, in_maps, list(range(2 * B))).results
    y = np.empty((B, S, D), dtype=np.float32)
    for b in range(B):
        y[b] = res[2 * b]["y"] + res[2 * b + 1]["y"]
    return y


# revision 19
# speedup vs baseline: 1.3264x; 1.2731x over previous
"""Linear attention (B=4, S=4096, D=1024, H=16) on 8 TRN2 NeuronCores.

Sharding: core = (batch, head-half): each core handles one batch's 8 heads.
 - x is host-transposed to xT [D, S] per batch so both operand orientations
   of every matmul come out of the tensor engine with no on-device transpose.
 - Wqkv column-sharded per head-half; Wo row-sharded; host sums the two
   partial y's per batch (row-parallel unshard).

Per-core dataflow (S=4096 in 8 blocks of 512 tokens):
  phase A: qkv projection (fp32r):
      QT [512f, S] feature-major  (lhsT=Wq, rhs=xT)   -> elu+1 -> bf16 QT
      K,V [S, 512f] token-major   (lhsT=xT, rhs=Wkv)  -> elu+1(K), copy(V) bf16
  phase B: per head-pair KV accumulation in PSUM over all tokens (bf16):
      kv_psum[p][:,0:128]  += K_pair^T @ V_pair
      kv_psum[p][:,128]    += K_pair^T @ ones      (= K_sum^T)
  phase C: per head h: psum[0:65,s] = [KV_h | Ksum_h]^T-fold @ QT_h
      row 64 = normalizer; rcp = 1/(norm+eps) (ACT); rcpb = ones x rcp (PE);
      outT = psum[0:64] * rcpb (DVE, bf16)   [feature-major out]
  phase D: y[s,:] (+)= outT^T @ Wo  (bf16 matmul, fp32 out)
"""

import numpy as np

import concourse.bacc as bacc
import concourse.mybir as mybir
import concourse.tile as tile
from concourse.bass_utils import run_bass_kernel_spmd

F32 = mybir.dt.float32
F32R = mybir.dt.float32r
BF16 = mybir.dt.bfloat16

P = 128
B, S, D = 4, 4096, 1024
H = 16
HD = 64
EPS = 1e-6

FSH = 512            # features per core for each of Q, K, V (8 heads)
KSUB = D // P        # 8 contraction subtiles
SBLK = 512           # tokens per block
NBLK = S // SBLK     # 8 blocks
TSUB = SBLK // P     # 4 token subtiles per block
NPAIR = 4            # head pairs per core
NHEAD = 8            # heads per core

_NC_CACHE = None


def build():
    nc = bacc.Bacc(target_bir_lowering=False)
    xT = nc.dram_tensor("xT", [D, S], F32, kind="ExternalInput")
    wqkv = nc.dram_tensor("wqkv", [D, 3 * FSH], F32, kind="ExternalInput")
    wo = nc.dram_tensor("wo", [FSH, D], BF16, kind="ExternalInput")
    y = nc.dram_tensor("y", [S, D], F32, kind="ExternalOutput")

    xT_r = xT.rearrange("(ko p) s -> p ko s", p=P)        # [128, 8, 4096]
    wqkv_r = wqkv.rearrange("(ko p) f -> p ko f", p=P)    # [128, 8, 1536]
    wo_r = wo.rearrange("(fo p) n -> p fo n", p=P)        # [128, 4, 1024]
    y_r = y.rearrange("(j t p) n -> j p t n", t=TSUB, p=P)  # [8, 128, 4, 1024]

    with tile.TileContext(nc) as tc:
        import contextlib

        with contextlib.ExitStack() as ctx:
            const = ctx.enter_context(tc.tile_pool(name="const", bufs=1))
            wpool = ctx.enter_context(tc.tile_pool(name="wpool", bufs=1))
            qtpool = ctx.enter_context(tc.tile_pool(name="qtpool", bufs=1))

            # persistent SBUF
            wqkv_sb = wpool.tile([P, KSUB, 3 * FSH], F32R)
            nc.sync.dma_start(out=wqkv_sb, in_=wqkv_r.bitcast(F32R))
            wo_sb = wpool.tile([P, FSH // P, D], BF16)
            nc.sync.dma_start(out=wo_sb, in_=wo_r)
            qt_sb = qtpool.tile([P, FSH // P, S], BF16)   # feature-major Q
            lhsT_sb = qtpool.tile([P, NHEAD, HD + 1], BF16)
            # norm-path scratch (only partition row 64 is used; one buf each)
            nrmln_sb = qtpool.tile([P, NHEAD, SBLK], F32)
            rcp_sb = qtpool.tile([P, NHEAD, SBLK], F32R)

            eps_sb = const.tile([P, 1], F32)
            nc.vector.memset(eps_sb, EPS)
            ones_f32 = const.tile([P, HD], F32)
            nc.vector.memset(ones_f32, 1.0)
            ones_fr = const.tile([P, HD], F32R)
            with nc.allow_low_precision(reason="fp32r ones constant"):
                nc.vector.tensor_copy(out=ones_fr, in_=ones_f32)

            # ---------------- phase A + B ----------------
            with (
                tc.tile_pool(name="kvps", bufs=1, space="PSUM") as kvps_pool,
                tc.tile_pool(name="xin", bufs=2) as xpool,
                tc.tile_pool(name="stage", bufs=2) as stpool,
                tc.tile_pool(name="paps", bufs=3, space="PSUM") as pa_ps,
                tc.tile_pool(name="etmp", bufs=3) as etpool,
            ):
                kvps = [
                    kvps_pool.tile([P, P + 1], F32, tag=f"kv{p}", name=f"kv{p}")
                    for p in range(NPAIR)
                ]

                for j in range(NBLK):
                    xt = xpool.tile([P, KSUB, SBLK], F32R, tag="xt")
                    nc.sync.dma_start(
                        out=xt,
                        in_=xT_r[:, :, j * SBLK : (j + 1) * SBLK].bitcast(F32R),
                    )

                    # QT: 4 feature blocks of 128
                    for f in range(FSH // P):
                        ps = pa_ps.tile([P, SBLK], F32, tag="pa")
                        for k in range(KSUB):
                            nc.tensor.matmul(
                                ps,
                                wqkv_sb[:, k, f * P : (f + 1) * P],
                                xt[:, k, :],
                                start=(k == 0),
                                stop=(k == KSUB - 1),
                            )
                        # elu(x)+1 = min(exp(x),1) + relu(x)
                        e = etpool.tile([P, SBLK], F32, tag="e")
                        nc.scalar.activation(
                            out=e, in_=ps, func=mybir.ActivationFunctionType.Exp
                        )
                        r = etpool.tile([P, SBLK], F32, tag="r")
                        nc.vector.tensor_scalar_max(r, ps, 0.0)
                        nc.vector.scalar_tensor_tensor(
                            out=qt_sb[:, f, j * SBLK : (j + 1) * SBLK],
                            in0=e,
                            scalar=1.0,
                            in1=r,
                            op0=mybir.AluOpType.min,
                            op1=mybir.AluOpType.add,
                        )

                    # K, V token-major per 128-token subtile.
                    # vst carries a ones column per head-pair slot so one
                    # matmul accumulates both KV and K_sum^T.
                    kst = stpool.tile([P, TSUB, FSH], BF16, tag="kst")
                    vst = stpool.tile([P, TSUB, NPAIR, P + 1], BF16, tag="vst")
                    nc.vector.memset(vst[:, :, :, P : P + 1], 1.0)
                    for t in range(TSUB):
                        psk = pa_ps.tile([P, FSH], F32, tag="pa")
                        for k in range(KSUB):
                            nc.tensor.matmul(
                                psk,
                                xt[:, k, t * P : (t + 1) * P],
                                wqkv_sb[:, k, FSH : 2 * FSH],
                                start=(k == 0),
                                stop=(k == KSUB - 1),
                            )
                        e = etpool.tile([P, SBLK], F32, tag="e")
                        nc.scalar.activation(
                            out=e, in_=psk, func=mybir.ActivationFunctionType.Exp
                        )
                        r = etpool.tile([P, SBLK], F32, tag="r")
                        nc.vector.tensor_scalar_max(r, psk, 0.0)
                        nc.vector.scalar_tensor_tensor(
                            out=kst[:, t, :],
                            in0=e,
                            scalar=1.0,
                            in1=r,
                            op0=mybir.AluOpType.min,
                            op1=mybir.AluOpType.add,
                        )

                        psv = pa_ps.tile([P, FSH], F32, tag="pa")
                        for k in range(KSUB):
                            nc.tensor.matmul(
                                psv,
                                xt[:, k, t * P : (t + 1) * P],
                                wqkv_sb[:, k, 2 * FSH : 3 * FSH],
                                start=(k == 0),
                                stop=(k == KSUB - 1),
                            )
                        nc.scalar.copy(out=vst[:, t, :, 0:P], in_=psv)

                    # phase B: accumulate [KV | K_sum^T] into persistent psums
                    first = j == 0
                    last = j == NBLK - 1
                    for t in range(TSUB):
                        for p_ in range(NPAIR):
                            nc.tensor.matmul(
                                kvps[p_],
                                kst[:, t, p_ * P : (p_ + 1) * P],
                                vst[:, t, p_, :],
                                start=(first and t == 0),
                                stop=(last and t == TSUB - 1),
                            )

                # assemble per-head lhsT = [KV_h | Ksum_h] (bf16)
                for h in range(NHEAD):
                    p_ = h // 2
                    base = (h % 2) * HD
                    nc.vector.tensor_copy(
                        out=lhsT_sb[base : base + HD, h, 0:HD],
                        in_=kvps[p_][base : base + HD, base : base + HD],
                    )
                    nc.vector.tensor_copy(
                        out=lhsT_sb[base : base + HD, h, HD : HD + 1],
                        in_=kvps[p_][base : base + HD, P : P + 1],
                    )

            # ---------------- phase C + D ----------------
            with (
                tc.tile_pool(name="pcps", bufs=3, space="PSUM") as pc_ps,
                tc.tile_pool(name="prps", bufs=3, space="PSUM") as pr_ps,
                tc.tile_pool(name="pyps", bufs=2, space="PSUM") as py_ps,
                tc.tile_pool(name="cd", bufs=2) as cdpool,
                tc.tile_pool(name="ou", bufs=1) as oupool,
                tc.tile_pool(name="yout", bufs=2) as ypool,
            ):
                for j in range(NBLK):
                    # C-1: per-head out+norm matmul, evicted unnormalized to
                    # SBUF fp32 (row 64 of each head slot = normalizer row)
                    outu = oupool.tile([P, NHEAD, SBLK], F32, tag="outu")
                    for h in range(NHEAD):
                        base = (h % 2) * HD
                        psc = pc_ps.tile([P, SBLK], F32, tag="pc")
                        nc.tensor.matmul(
                            psc[0 : HD + 1, :],
                            lhsT_sb[base : base + HD, h, :],
                            qt_sb[base : base + HD, h // 2, j * SBLK : (j + 1) * SBLK],
                            start=True,
                            stop=True,
                        )
                        nc.vector.tensor_copy(
                            out=outu[0 : HD + 1, h, :], in_=psc[0 : HD + 1, :]
                        )

                    # C-2: rcp = exp(-ln(norm+eps)) on ACT, batched per head
                    # pair ([1, 1024] per op) to amortize table switching
                    for p_ in range(NPAIR):
                        nc.scalar.activation(
                            out=nrmln_sb[HD : HD + 1, 2 * p_ : 2 * p_ + 2, :],
                            in_=outu[HD : HD + 1, 2 * p_ : 2 * p_ + 2, :],
                            func=mybir.ActivationFunctionType.Ln,
                            bias=eps_sb[HD : HD + 1, :],
                        )
                    for p_ in range(NPAIR):
                        with nc.allow_low_precision(
                            reason="fp32r is 32-bit; needed as fp32r matmul operand"
                        ):
                            nc.scalar.activation(
                                out=rcp_sb[HD : HD + 1, 2 * p_ : 2 * p_ + 2, :],
                                in_=nrmln_sb[HD : HD + 1, 2 * p_ : 2 * p_ + 2, :],
                                func=mybir.ActivationFunctionType.Exp,
                                scale=-1.0,
                            )

                    # C-3: broadcast rcp across 64 partitions via PE outer
                    # product; apply straight from PSUM on DVE (bf16 out)
                    outt = cdpool.tile([P, FSH // P, SBLK], BF16, tag="outt")
                    for h in range(NHEAD):
                        base = (h % 2) * HD
                        psr = pr_ps.tile([P, SBLK], F32, tag="pr")
                        nc.tensor.matmul(
                            psr[0:HD, :],
                            ones_fr[HD : HD + 1, 0:HD],
                            rcp_sb[HD : HD + 1, h, :],
                            start=True,
                            stop=True,
                        )
                        nc.vector.tensor_tensor(
                            out=outt[base : base + HD, h // 2, :],
                            in0=outu[0:HD, h, :],
                            in1=psr[0:HD, :],
                            op=mybir.AluOpType.mult,
                        )

                    # phase D for this block
                    ysb = ypool.tile([P, TSUB, D], F32, tag="ysb")
                    for t in range(TSUB):
                        for nb in range(D // 512):
                            psy = py_ps.tile([P, 512], F32, tag="py")
                            for fs in range(FSH // P):
                                nc.tensor.matmul(
                                    psy,
                                    outt[:, fs, t * P : (t + 1) * P],
                                    wo_sb[:, fs, nb * 512 : (nb + 1) * 512],
                                    start=(fs == 0),
                                    stop=(fs == FSH // P - 1),
                                )
                            nc.vector.tensor_copy(
                                out=ysb[:, t, nb * 512 : (nb + 1) * 512], in_=psy
                            )
                    nc.sync.dma_start(out=y_r[j], in_=ysb)

    nc.compile()
    return nc


def _prep_inputs(x, Wqkv, Wo):
    import ml_dtypes

    x = np.ascontiguousarray(x, dtype=np.float32)
    Wqkv = np.ascontiguousarray(Wqkv, dtype=np.float32)
    Wo = np.ascontiguousarray(Wo, dtype=np.float32)
    in_maps = []
    for b in range(B):
        xT = np.ascontiguousarray(x[b].T)  # [D, S]
        for hh in range(2):
            cols = slice(hh * FSH, (hh + 1) * FSH)
            wq = Wqkv[:, 0 * D :][:, cols]
            wk = Wqkv[:, 1 * D :][:, cols]
            wv = Wqkv[:, 2 * D :][:, cols]
            wqkv_sh = np.ascontiguousarray(np.concatenate([wq, wk, wv], axis=1))
            wo_sh = np.ascontiguousarray(Wo[hh * FSH : (hh + 1) * FSH, :]).astype(
                ml_dtypes.bfloat16
            )
            in_maps.append({"xT": xT, "wqkv": wqkv_sh, "wo": wo_sh})
    return in_maps


def kernel(x, Wqkv, Wo):
    global _NC_CACHE
    if _NC_CACHE is None:
        _NC_CACHE = build()
    nc = _NC_CACHE
    in_maps = _prep_inputs(x, Wqkv, Wo)
    res = run_bass_kernel_spmd(nc, in_maps, list(range(2 * B))).results
    y = np.empty((B, S, D), dtype=np.float32)
    for b in range(B):
        y[b] = res[2 * b]["y"] + res[2 * b + 1]["y"]
    return y


# revision 31
# speedup vs baseline: 1.6257x; 1.2256x over previous
"""Linear attention (B=4, S=4096, D=1024, H=16) on 8 TRN2 NeuronCores.

Sharding: core = (batch, head-half): each core handles one batch's 8 heads.
 - x is host-transposed to xT [D, S] per batch so both operand orientations
   of every matmul come out of the tensor engine with no on-device transpose.
 - Wqkv column-sharded per head-half; Wo row-sharded; host sums the two
   partial y's per batch (row-parallel unshard).

Per-core dataflow (S=4096 in 8 blocks of 512 tokens):
  phase A: qkv projection (fp32r):
      QT [512f, S] feature-major  (lhsT=Wq, rhs=xT)   -> elu+1 -> bf16 QT
      K,V [S, 512f] token-major   (lhsT=xT, rhs=Wkv)  -> elu+1(K), copy(V) bf16
  phase B: per head-pair KV accumulation in PSUM over all tokens (bf16):
      kv_psum[p][:,0:128]  += K_pair^T @ V_pair
      kv_psum[p][:,128]    += K_pair^T @ ones      (= K_sum^T)
  phase C: per head h: psum[0:65,s] = [KV_h | Ksum_h]^T-fold @ QT_h
      row 64 = normalizer; rcp = 1/(norm+eps) (ACT); rcpb = ones x rcp (PE);
      outT = psum[0:64] * rcpb (DVE, bf16)   [feature-major out]
  phase D: y[s,:] (+)= outT^T @ Wo  (bf16 matmul, fp32 out)
"""

import numpy as np

import concourse.bacc as bacc
import concourse.mybir as mybir
import concourse.tile as tile
from concourse.bass_utils import run_bass_kernel_spmd

F32 = mybir.dt.float32
F32R = mybir.dt.float32r
BF16 = mybir.dt.bfloat16

P = 128
B, S, D = 4, 4096, 1024
H = 16
HD = 64
EPS = 1e-6

FSH = 512            # features per core for each of Q, K, V (8 heads)
KSUB = D // P        # 8 contraction subtiles
SBLK = 512           # tokens per block
NBLK = S // SBLK     # 8 blocks
TSUB = SBLK // P     # 4 token subtiles per block
NPAIR = 4            # head pairs per core
NHEAD = 8            # heads per core

_NC_CACHE = None


def build():
    nc = bacc.Bacc(target_bir_lowering=False)
    xT = nc.dram_tensor("xT", [D, S], BF16, kind="ExternalInput")
    wqkv = nc.dram_tensor("wqkv", [D, 3 * FSH], BF16, kind="ExternalInput")
    wo = nc.dram_tensor("wo", [FSH, D], BF16, kind="ExternalInput")
    y = nc.dram_tensor("y", [S, D], F32, kind="ExternalOutput")

    xT_r = xT.rearrange("(ko p) s -> p ko s", p=P)        # [128, 8, 4096]
    wqkv_r = wqkv.rearrange("(ko p) f -> p ko f", p=P)    # [128, 8, 1536]
    wo_r = wo.rearrange("(fo p) n -> p fo n", p=P)        # [128, 4, 1024]
    y_r2 = y.rearrange(
        "(j th t p) n -> j p th t n", th=TSUB // 2, t=2, p=P
    )  # [8, 128, 2, 2, 1024]

    with tile.TileContext(nc) as tc:
        import contextlib

        with contextlib.ExitStack() as ctx:
            const = ctx.enter_context(tc.tile_pool(name="const", bufs=1))
            wpool = ctx.enter_context(tc.tile_pool(name="wpool", bufs=1))
            qtpool = ctx.enter_context(tc.tile_pool(name="qtpool", bufs=1))

            # persistent SBUF
            wqkv_sb = wpool.tile([P, KSUB, 3 * FSH], BF16)
            nc.sync.dma_start(out=wqkv_sb, in_=wqkv_r)
            wo_sb = wpool.tile([P, FSH // P, D], BF16)
            nc.sync.dma_start(out=wo_sb, in_=wo_r)
            qt_sb = qtpool.tile([P, FSH // P, S], BF16)   # feature-major Q
            lhsT_sb = qtpool.tile([P, NHEAD, HD + 1], BF16)
            ksumpad_sb = qtpool.tile([P, NHEAD, 32], BF16)
            # norm-path scratch (only partition row 64 is used; one buf each)
            nrmln_sb = qtpool.tile([P, NHEAD, SBLK], F32)

            eps_sb = const.tile([P, 1], F32)
            nc.vector.memset(eps_sb, EPS)
            ones_f32 = const.tile([P, HD], F32)
            nc.vector.memset(ones_f32, 1.0)
            ones_fr = const.tile([P, HD], F32R)
            with nc.allow_low_precision(reason="fp32r ones constant"):
                nc.vector.tensor_copy(out=ones_fr, in_=ones_f32)

            # ---------------- phase A + B ----------------
            with (
                tc.tile_pool(name="kvps", bufs=1, space="PSUM") as kvps_pool,
                tc.tile_pool(name="xin", bufs=3) as xpool,
                tc.tile_pool(name="stage", bufs=3) as stpool,
                tc.tile_pool(name="paps", bufs=4, space="PSUM") as pa_ps,
                tc.tile_pool(name="etmp", bufs=4) as etpool,
            ):
                kvps = [
                    kvps_pool.tile([P, P + 1], F32, tag=f"kv{p}", name=f"kv{p}")
                    for p in range(NPAIR)
                ]

                for j in range(NBLK):
                    xt = xpool.tile([P, KSUB, SBLK], BF16, tag="xt")
                    nc.sync.dma_start(
                        out=xt, in_=xT_r[:, :, j * SBLK : (j + 1) * SBLK]
                    )

                    # QT: 4 feature blocks of 128
                    for f in range(FSH // P):
                        ps = pa_ps.tile([P, SBLK], F32, tag="pa")
                        for k in range(KSUB):
                            nc.tensor.matmul(
                                ps,
                                wqkv_sb[:, k, f * P : (f + 1) * P],
                                xt[:, k, :],
                                start=(k == 0),
                                stop=(k == KSUB - 1),
                            )
                        e = etpool.tile([P, SBLK], F32, tag="e")
                        nc.scalar.activation(
                            out=e, in_=ps, func=mybir.ActivationFunctionType.Exp
                        )
                        r = etpool.tile([P, SBLK], F32, tag="r")
                        nc.vector.tensor_scalar_max(r, ps, 0.0)
                        nc.vector.scalar_tensor_tensor(
                            out=qt_sb[:, f, j * SBLK : (j + 1) * SBLK],
                            in0=e,
                            scalar=1.0,
                            in1=r,
                            op0=mybir.AluOpType.min,
                            op1=mybir.AluOpType.add,
                        )

                    # K, V token-major per 128-token subtile.
                    # vst carries a ones column per head-pair slot so one
                    # matmul accumulates both KV and K_sum^T.
                    kst = stpool.tile([P, TSUB, FSH], BF16, tag="kst")
                    vst = stpool.tile([P, TSUB, NPAIR, P + 1], BF16, tag="vst")
                    nc.vector.memset(vst[:, :, :, P : P + 1], 1.0)
                    for t in range(TSUB):
                        psk = pa_ps.tile([P, FSH], F32, tag="pa")
                        psv = pa_ps.tile([P, FSH], F32, tag="pa")
                        for k in range(KSUB):
                            nc.tensor.matmul(
                                psk,
                                xt[:, k, t * P : (t + 1) * P],
                                wqkv_sb[:, k, FSH : 2 * FSH],
                                start=(k == 0),
                                stop=(k == KSUB - 1),
                            )
                            nc.tensor.matmul(
                                psv,
                                xt[:, k, t * P : (t + 1) * P],
                                wqkv_sb[:, k, 2 * FSH : 3 * FSH],
                                start=(k == 0),
                                stop=(k == KSUB - 1),
                            )
                        e = etpool.tile([P, SBLK], F32, tag="e")
                        nc.scalar.activation(
                            out=e, in_=psk, func=mybir.ActivationFunctionType.Exp
                        )
                        r = etpool.tile([P, SBLK], F32, tag="r")
                        nc.vector.tensor_scalar_max(r, psk, 0.0)
                        nc.vector.scalar_tensor_tensor(
                            out=kst[:, t, :],
                            in0=e,
                            scalar=1.0,
                            in1=r,
                            op0=mybir.AluOpType.min,
                            op1=mybir.AluOpType.add,
                        )

                        nc.scalar.copy(out=vst[:, t, :, 0:P], in_=psv)

                    # phase B: accumulate [KV | K_sum^T] into persistent psums
                    first = j == 0
                    last = j == NBLK - 1
                    for t in range(TSUB):
                        for p_ in range(NPAIR):
                            nc.tensor.matmul(
                                kvps[p_],
                                kst[:, t, p_ * P : (p_ + 1) * P],
                                vst[:, t, p_, :],
                                start=(first and t == 0),
                                stop=(last and t == TSUB - 1),
                            )

                # ksumpad: per-head [Ksum_h | zeros] (64 x 32) so the norm
                # matmuls write full 32-row blocks (zeros elsewhere)
                nc.vector.memset(ksumpad_sb, 0.0)
                for h in range(NHEAD):
                    p_ = h // 2
                    base = (h % 2) * HD
                    nc.vector.tensor_copy(
                        out=ksumpad_sb[base : base + HD, h, 0:1],
                        in_=kvps[p_][base : base + HD, P : P + 1],
                    )
                # assemble per-head lhsT = [KV_h | Ksum_h] (bf16)
                for h in range(NHEAD):
                    p_ = h // 2
                    base = (h % 2) * HD
                    nc.vector.tensor_copy(
                        out=lhsT_sb[base : base + HD, h, 0:HD],
                        in_=kvps[p_][base : base + HD, base : base + HD],
                    )
                    nc.vector.tensor_copy(
                        out=lhsT_sb[base : base + HD, h, HD : HD + 1],
                        in_=kvps[p_][base : base + HD, P : P + 1],
                    )

            # ---------------- phase C + D ----------------
            # software-pipelined: C1(j) runs ahead while C3/D(j-1) finish.
            # Normalizer rows are computed by separate M=32 zero-padded
            # matmuls so all 8 land on 32-aligned partitions of 2 PSUM banks
            # -> Ln/Exp run 4 lanes wide instead of 1.
            with (
                tc.tile_pool(name="pcps", bufs=2, space="PSUM") as pc_ps,
                tc.tile_pool(name="pnps", bufs=1, space="PSUM") as pn_ps,
                tc.tile_pool(name="prps", bufs=2, space="PSUM") as pr_ps,
                tc.tile_pool(name="pyps", bufs=2, space="PSUM") as py_ps,
                tc.tile_pool(name="cd", bufs=2) as cdpool,
                tc.tile_pool(name="ou", bufs=2) as oupool,
                tc.tile_pool(name="rc", bufs=2) as rcpool,
                tc.tile_pool(name="yout", bufs=2) as ypool,
            ):
                outus = {}
                rcps = {}

                def phase_c1(j):
                    outu = oupool.tile([P, NHEAD, SBLK], F32, tag="outu", name="outu")
                    outus[j] = outu
                    for h in range(NHEAD):
                        base = (h % 2) * HD
                        psc = pc_ps.tile([P, SBLK], F32, tag="pc", name="psc")
                        nc.tensor.matmul(
                            psc[0:HD, :],
                            lhsT_sb[base : base + HD, h, 0:HD],
                            qt_sb[base : base + HD, h // 2, j * SBLK : (j + 1) * SBLK],
                            start=True,
                            stop=True,
                        )
                        nc.scalar.copy(out=outu[0:HD, h, :], in_=psc[0:HD, :])
                    # normalizer rows -> psn banks (rows 32k = head norms)
                    psns = [
                        pn_ps.tile([P, SBLK], F32, tag=f"pn{b}", name=f"psn{b}")
                        for b in range(2)
                    ]
                    for h in range(NHEAD):
                        base = (h % 2) * HD
                        nc.tensor.matmul(
                            psns[h // 4][32 * (h % 4) : 32 * (h % 4) + 32, :],
                            ksumpad_sb[base : base + HD, h, :],
                            qt_sb[base : base + HD, h // 2, j * SBLK : (j + 1) * SBLK],
                            start=True,
                            stop=True,
                            tile_position=(base, 32 * (h % 4)),
                        )
                    # rcp = exp(-ln(norm+eps)) on ACT, full-bank ops
                    nrmts = []
                    for b in range(2):
                        nrmt = rcpool.tile([P, SBLK], F32, tag=f"nt{b}", name="nrmt")
                        nc.scalar.activation(
                            out=nrmt,
                            in_=psns[b],
                            func=mybir.ActivationFunctionType.Ln,
                            bias=eps_sb,
                        )
                        nrmts.append(nrmt)
                    rcpb_ts = []
                    for b in range(2):
                        rcpt = rcpool.tile([P, SBLK], F32R, tag=f"rc{b}", name="rcpt")
                        with nc.allow_low_precision(
                            reason="fp32r is 32-bit; fp32r matmul operand"
                        ):
                            nc.scalar.activation(
                                out=rcpt,
                                in_=nrmts[b],
                                func=mybir.ActivationFunctionType.Exp,
                                scale=-1.0,
                            )
                        rcpb_ts.append(rcpt)
                    rcps[j] = rcpb_ts

                def phase_c3_d(j):
                    outu = outus.pop(j)
                    rcpts = rcps.pop(j)
                    outt = cdpool.tile(
                        [P, FSH // P, SBLK], BF16, tag="outt", name="outt"
                    )
                    for h in range(NHEAD):
                        base = (h % 2) * HD
                        rb = 32 * (h % 4)
                        psr = pr_ps.tile([P, SBLK], F32, tag="pr", name="psr")
                        nc.tensor.matmul(
                            psr[0:HD, :],
                            ones_fr[rb : rb + 1, 0:HD],
                            rcpts[h // 4][rb : rb + 1, :],
                            start=True,
                            stop=True,
                            tile_position=(rb, 0),
                        )
                        nc.vector.tensor_tensor(
                            out=outt[base : base + HD, h // 2, :],
                            in0=outu[0:HD, h, :],
                            in1=psr[0:HD, :],
                            op=mybir.AluOpType.mult,
                        )
                    for th in range(TSUB // 2):
                        ysb = ypool.tile([P, 2, D], F32, tag="ysb", name="ysb")
                        for t2 in range(2):
                            t = th * 2 + t2
                            for nb in range(D // 512):
                                psy = py_ps.tile([P, 512], F32, tag="py", name="psy")
                                for fs in range(FSH // P):
                                    nc.tensor.matmul(
                                        psy,
                                        outt[:, fs, t * P : (t + 1) * P],
                                        wo_sb[:, fs, nb * 512 : (nb + 1) * 512],
                                        start=(fs == 0),
                                        stop=(fs == FSH // P - 1),
                                    )
                                nc.vector.tensor_copy(
                                    out=ysb[:, t2, nb * 512 : (nb + 1) * 512], in_=psy
                                )
                        nc.sync.dma_start(out=y_r2[j, :, th], in_=ysb)

                for j in range(NBLK):
                    phase_c1(j)
                    if j >= 1:
                        phase_c3_d(j - 1)
                phase_c3_d(NBLK - 1)

    nc.compile()
    return nc


def _prep_inputs(x, Wqkv, Wo):
    import ml_dtypes

    x = np.ascontiguousarray(x, dtype=np.float32)
    Wqkv = np.ascontiguousarray(Wqkv, dtype=np.float32)
    Wo = np.ascontiguousarray(Wo, dtype=np.float32)
    in_maps = []
    for b in range(B):
        xT = np.ascontiguousarray(x[b].T).astype(ml_dtypes.bfloat16)  # [D, S]
        for hh in range(2):
            cols = slice(hh * FSH, (hh + 1) * FSH)
            wq = Wqkv[:, 0 * D :][:, cols]
            wk = Wqkv[:, 1 * D :][:, cols]
            wv = Wqkv[:, 2 * D :][:, cols]
            wqkv_sh = np.ascontiguousarray(
                np.concatenate([wq, wk, wv], axis=1)
            ).astype(ml_dtypes.bfloat16)
            wo_sh = np.ascontiguousarray(Wo[hh * FSH : (hh + 1) * FSH, :]).astype(
                ml_dtypes.bfloat16
            )
            in_maps.append({"xT": xT, "wqkv": wqkv_sh, "wo": wo_sh})
    return in_maps


def kernel(x, Wqkv, Wo):
    global _NC_CACHE
    if _NC_CACHE is None:
        _NC_CACHE = build()
    nc = _NC_CACHE
    in_maps = _prep_inputs(x, Wqkv, Wo)
    res = run_bass_kernel_spmd(nc, in_maps, list(range(2 * B))).results
    y = np.empty((B, S, D), dtype=np.float32)
    for b in range(B):
        y[b] = res[2 * b]["y"] + res[2 * b + 1]["y"]
    return y


# revision 34
# speedup vs baseline: 1.7308x; 1.0647x over previous
"""Linear attention (B=4, S=4096, D=1024, H=16) on 8 TRN2 NeuronCores.

Sharding: core = (batch, head-half): each core handles one batch's 8 heads.
 - x is host-transposed to xT [D, S] per batch so both operand orientations
   of every matmul come out of the tensor engine with no on-device transpose.
 - Wqkv column-sharded per head-half; Wo row-sharded; host sums the two
   partial y's per batch (row-parallel unshard).

Per-core dataflow (S=4096 in 8 blocks of 512 tokens):
  phase A: qkv projection (fp32r):
      QT [512f, S] feature-major  (lhsT=Wq, rhs=xT)   -> elu+1 -> bf16 QT
      K,V [S, 512f] token-major   (lhsT=xT, rhs=Wkv)  -> elu+1(K), copy(V) bf16
  phase B: per head-pair KV accumulation in PSUM over all tokens (bf16):
      kv_psum[p][:,0:128]  += K_pair^T @ V_pair
      kv_psum[p][:,128]    += K_pair^T @ ones      (= K_sum^T)
  phase C: per head h: psum[0:65,s] = [KV_h | Ksum_h]^T-fold @ QT_h
      row 64 = normalizer; rcp = 1/(norm+eps) (ACT); rcpb = ones x rcp (PE);
      outT = psum[0:64] * rcpb (DVE, bf16)   [feature-major out]
  phase D: y[s,:] (+)= outT^T @ Wo  (bf16 matmul, fp32 out)
"""

import numpy as np

import concourse.bacc as bacc
import concourse.mybir as mybir
import concourse.tile as tile
from concourse.bass_utils import run_bass_kernel_spmd

F32 = mybir.dt.float32
F32R = mybir.dt.float32r
BF16 = mybir.dt.bfloat16

P = 128
B, S, D = 4, 4096, 1024
H = 16
HD = 64
EPS = 1e-6

FSH = 512            # features per core for each of Q, K, V (8 heads)
KSUB = D // P        # 8 contraction subtiles
SBLK = 512           # tokens per block
NBLK = S // SBLK     # 8 blocks
TSUB = SBLK // P     # 4 token subtiles per block
NPAIR = 4            # head pairs per core
NHEAD = 8            # heads per core

_NC_CACHE = None


def build():
    nc = bacc.Bacc(target_bir_lowering=False)
    xT = nc.dram_tensor("xT", [D, S], BF16, kind="ExternalInput")
    wqkv = nc.dram_tensor("wqkv", [D, 3 * FSH], BF16, kind="ExternalInput")
    wo = nc.dram_tensor("wo", [FSH, D], BF16, kind="ExternalInput")
    ones2 = nc.dram_tensor("ones2", [P, P], F32R, kind="ExternalInput")
    y = nc.dram_tensor("y", [S, D], F32, kind="ExternalOutput")

    xT_r = xT.rearrange("(ko p) s -> p ko s", p=P)        # [128, 8, 4096]
    wqkv_r = wqkv.rearrange("(ko p) f -> p ko f", p=P)    # [128, 8, 1536]
    wo_r = wo.rearrange("(fo p) n -> p fo n", p=P)        # [128, 4, 1024]
    y_r2 = y.rearrange(
        "(j th t p) n -> j p th t n", th=TSUB // 2, t=2, p=P
    )  # [8, 128, 2, 2, 1024]

    with tile.TileContext(nc) as tc:
        import contextlib

        with contextlib.ExitStack() as ctx:
            const = ctx.enter_context(tc.tile_pool(name="const", bufs=1))
            wpool = ctx.enter_context(tc.tile_pool(name="wpool", bufs=1))
            qtpool = ctx.enter_context(tc.tile_pool(name="qtpool", bufs=1))

            # persistent SBUF
            wqkv_sb = wpool.tile([P, KSUB, 3 * FSH], BF16)
            nc.sync.dma_start(out=wqkv_sb, in_=wqkv_r)
            wo_sb = wpool.tile([P, FSH // P, D], BF16)
            nc.sync.dma_start(out=wo_sb, in_=wo_r)
            qt_sb = qtpool.tile([P, FSH // P, S], BF16)   # feature-major Q
            lhsT_sb = [
                qtpool.tile([P, HD], BF16, name=f"lhsT{h}") for h in range(NHEAD)
            ]
            # per-pair [Ksum_h0 | Ksum_h1 | zeros] (128 x 32): col 0 rows 0:64
            # = Ksum_even, col 1 rows 64:128 = Ksum_odd
            ksumpad_sb = [
                qtpool.tile([P, 32], BF16, name=f"ksp{p}") for p in range(NPAIR)
            ]
            # norm-path scratch (only partition row 64 is used; one buf each)
            nrmln_sb = qtpool.tile([P, NHEAD, SBLK], F32)

            eps_sb = const.tile([P, 1], F32)
            nc.vector.memset(eps_sb, EPS)
            # ones2 (host-built): per 32-block, row 32k = [1x64 | 0x64],
            # row 32k+1 = [0x64 | 1x64] -- pair-broadcast stationary operand
            ones2_fr = const.tile([P, P], F32R)
            nc.sync.dma_start(out=ones2_fr, in_=ones2[:])

            # ---------------- phase A + B ----------------
            with (
                tc.tile_pool(name="kvps", bufs=1, space="PSUM") as kvps_pool,
                tc.tile_pool(name="xin", bufs=3) as xpool,
                tc.tile_pool(name="stage", bufs=3) as stpool,
                tc.tile_pool(name="paps", bufs=4, space="PSUM") as pa_ps,
                tc.tile_pool(name="etmp", bufs=4) as etpool,
            ):
                kvps = [
                    kvps_pool.tile([P, P + 1], F32, tag=f"kv{p}", name=f"kv{p}")
                    for p in range(NPAIR)
                ]

                for j in range(NBLK):
                    xt = xpool.tile([P, KSUB, SBLK], BF16, tag="xt")
                    nc.sync.dma_start(
                        out=xt, in_=xT_r[:, :, j * SBLK : (j + 1) * SBLK]
                    )

                    # QT: 4 feature blocks of 128
                    for f in range(FSH // P):
                        ps = pa_ps.tile([P, SBLK], F32, tag="pa")
                        for k in range(KSUB):
                            nc.tensor.matmul(
                                ps,
                                wqkv_sb[:, k, f * P : (f + 1) * P],
                                xt[:, k, :],
                                start=(k == 0),
                                stop=(k == KSUB - 1),
                            )
                        e = etpool.tile([P, SBLK], F32, tag="e")
                        nc.scalar.activation(
                            out=e, in_=ps, func=mybir.ActivationFunctionType.Exp
                        )
                        r = etpool.tile([P, SBLK], F32, tag="r")
                        nc.vector.tensor_scalar_max(r, ps, 0.0)
                        nc.vector.scalar_tensor_tensor(
                            out=qt_sb[:, f, j * SBLK : (j + 1) * SBLK],
                            in0=e,
                            scalar=1.0,
                            in1=r,
                            op0=mybir.AluOpType.min,
                            op1=mybir.AluOpType.add,
                        )

                    # K, V token-major per 128-token subtile.
                    # vst carries a ones column per head-pair slot so one
                    # matmul accumulates both KV and K_sum^T.
                    kst = stpool.tile([P, TSUB, FSH], BF16, tag="kst")
                    vst = stpool.tile([P, TSUB, NPAIR, P + 1], BF16, tag="vst")
                    nc.vector.memset(vst[:, :, :, P : P + 1], 1.0)
                    for t in range(TSUB):
                        psk = pa_ps.tile([P, FSH], F32, tag="pa")
                        psv = pa_ps.tile([P, FSH], F32, tag="pa")
                        for k in range(KSUB):
                            nc.tensor.matmul(
                                psk,
                                xt[:, k, t * P : (t + 1) * P],
                                wqkv_sb[:, k, FSH : 2 * FSH],
                                start=(k == 0),
                                stop=(k == KSUB - 1),
                            )
                            nc.tensor.matmul(
                                psv,
                                xt[:, k, t * P : (t + 1) * P],
                                wqkv_sb[:, k, 2 * FSH : 3 * FSH],
                                start=(k == 0),
                                stop=(k == KSUB - 1),
                            )
                        e = etpool.tile([P, SBLK], F32, tag="e")
                        nc.scalar.activation(
                            out=e, in_=psk, func=mybir.ActivationFunctionType.Exp
                        )
                        r = etpool.tile([P, SBLK], F32, tag="r")
                        nc.vector.tensor_scalar_max(r, psk, 0.0)
                        nc.vector.scalar_tensor_tensor(
                            out=kst[:, t, :],
                            in0=e,
                            scalar=1.0,
                            in1=r,
                            op0=mybir.AluOpType.min,
                            op1=mybir.AluOpType.add,
                        )

                        nc.scalar.copy(out=vst[:, t, :, 0:P], in_=psv)

                    # phase B: accumulate [KV | K_sum^T] into persistent psums
                    first = j == 0
                    last = j == NBLK - 1
                    for t in range(TSUB):
                        for p_ in range(NPAIR):
                            nc.tensor.matmul(
                                kvps[p_],
                                kst[:, t, p_ * P : (p_ + 1) * P],
                                vst[:, t, p_, :],
                                start=(first and t == 0),
                                stop=(last and t == TSUB - 1),
                            )

                for p_ in range(NPAIR):
                    nc.vector.memset(ksumpad_sb[p_], 0.0)
                    nc.vector.tensor_copy(
                        out=ksumpad_sb[p_][0:HD, 0:1],
                        in_=kvps[p_][0:HD, P : P + 1],
                    )
                    nc.vector.tensor_copy(
                        out=ksumpad_sb[p_][HD:P, 1:2],
                        in_=kvps[p_][HD:P, P : P + 1],
                    )
                # per-head KV lhsT (bf16)
                for h in range(NHEAD):
                    p_ = h // 2
                    base = (h % 2) * HD
                    nc.vector.tensor_copy(
                        out=lhsT_sb[h][base : base + HD, :],
                        in_=kvps[p_][base : base + HD, base : base + HD],
                    )

            # ---------------- phase C + D ----------------
            # software-pipelined: C1(j) runs ahead while C3/D(j-1) finish.
            # Normalizer rows are computed by separate M=32 zero-padded
            # matmuls so all 8 land on 32-aligned partitions of 2 PSUM banks
            # -> Ln/Exp run 4 lanes wide instead of 1.
            with (
                tc.tile_pool(name="pcps", bufs=2, space="PSUM") as pc_ps,
                tc.tile_pool(name="pnps", bufs=1, space="PSUM") as pn_ps,
                tc.tile_pool(name="prps", bufs=2, space="PSUM") as pr_ps,
                tc.tile_pool(name="pyps", bufs=2, space="PSUM") as py_ps,
                tc.tile_pool(name="cd", bufs=2) as cdpool,
                tc.tile_pool(name="ou", bufs=2) as oupool,
                tc.tile_pool(name="rc", bufs=2) as rcpool,
                tc.tile_pool(name="yout", bufs=2) as ypool,
            ):
                outus = {}
                rcps = {}

                def phase_c1(j):
                    outu = oupool.tile([P, NHEAD, SBLK], F32, tag="outu", name="outu")
                    outus[j] = outu
                    for h in range(NHEAD):
                        base = (h % 2) * HD
                        psc = pc_ps.tile([P, SBLK], F32, tag="pc", name="psc")
                        nc.tensor.matmul(
                            psc[0:HD, :],
                            lhsT_sb[h][base : base + HD, :],
                            qt_sb[base : base + HD, h // 2, j * SBLK : (j + 1) * SBLK],
                            start=True,
                            stop=True,
                        )
                        nc.scalar.copy(out=outu[0:HD, h, :], in_=psc[0:HD, :])
                    # normalizer rows -> one psn bank: pair p_ rows 32p_,
                    # 32p_+1 (zero-padded M=32 matmuls)
                    psn = pn_ps.tile([P, SBLK], F32, tag="pn", name="psn")
                    for p_ in range(NPAIR):
                        nc.tensor.matmul(
                            psn[32 * p_ : 32 * p_ + 32, :],
                            ksumpad_sb[p_],
                            qt_sb[:, p_, j * SBLK : (j + 1) * SBLK],
                            start=True,
                            stop=True,
                            tile_position=(0, 32 * p_),
                        )
                    # rcp = exp(-ln(norm+eps)) on ACT, full-bank ops
                    nrmt = rcpool.tile([P, SBLK], F32, tag="nt", name="nrmt")
                    nc.scalar.activation(
                        out=nrmt,
                        in_=psn,
                        func=mybir.ActivationFunctionType.Ln,
                        bias=eps_sb,
                    )
                    rcpt = rcpool.tile([P, SBLK], F32R, tag="rc", name="rcpt")
                    with nc.allow_low_precision(
                        reason="fp32r is 32-bit; fp32r matmul operand"
                    ):
                        nc.scalar.activation(
                            out=rcpt,
                            in_=nrmt,
                            func=mybir.ActivationFunctionType.Exp,
                            scale=-1.0,
                        )
                    rcps[j] = rcpt

                def phase_c3_d(j):
                    outu = outus.pop(j)
                    rcpt = rcps.pop(j)
                    outt = cdpool.tile(
                        [P, FSH // P, SBLK], BF16, tag="outt", name="outt"
                    )
                    for p_ in range(NPAIR):
                        rb = 32 * p_
                        psr = pr_ps.tile([P, SBLK], F32, tag="pr", name="psr")
                        nc.tensor.matmul(
                            psr,
                            ones2_fr[rb : rb + 2, :],
                            rcpt[rb : rb + 2, :],
                            start=True,
                            stop=True,
                            tile_position=(rb, 0),
                        )
                        for c in range(2):
                            h = 2 * p_ + c
                            base = c * HD
                            nc.vector.tensor_tensor(
                                out=outt[base : base + HD, h // 2, :],
                                in0=outu[0:HD, h, :],
                                in1=psr[base : base + HD, :],
                                op=mybir.AluOpType.mult,
                            )
                    for th in range(TSUB // 2):
                        ysb = ypool.tile([P, 2, D], F32, tag="ysb", name="ysb")
                        for t2 in range(2):
                            t = th * 2 + t2
                            for nb in range(D // 512):
                                psy = py_ps.tile([P, 512], F32, tag="py", name="psy")
                                for fs in range(FSH // P):
                                    nc.tensor.matmul(
                                        psy,
                                        outt[:, fs, t * P : (t + 1) * P],
                                        wo_sb[:, fs, nb * 512 : (nb + 1) * 512],
                                        start=(fs == 0),
                                        stop=(fs == FSH // P - 1),
                                    )
                                nc.vector.tensor_copy(
                                    out=ysb[:, t2, nb * 512 : (nb + 1) * 512], in_=psy
                                )
                        nc.sync.dma_start(out=y_r2[j, :, th], in_=ysb)

                for j in range(NBLK):
                    phase_c1(j)
                    if j >= 1:
                        phase_c3_d(j - 1)
                phase_c3_d(NBLK - 1)

    nc.compile()
    return nc


def _prep_inputs(x, Wqkv, Wo):
    import ml_dtypes

    x = np.ascontiguousarray(x, dtype=np.float32)
    Wqkv = np.ascontiguousarray(Wqkv, dtype=np.float32)
    Wo = np.ascontiguousarray(Wo, dtype=np.float32)
    in_maps = []
    for b in range(B):
        xT = np.ascontiguousarray(x[b].T).astype(ml_dtypes.bfloat16)  # [D, S]
        for hh in range(2):
            cols = slice(hh * FSH, (hh + 1) * FSH)
            wq = Wqkv[:, 0 * D :][:, cols]
            wk = Wqkv[:, 1 * D :][:, cols]
            wv = Wqkv[:, 2 * D :][:, cols]
            wqkv_sh = np.ascontiguousarray(
                np.concatenate([wq, wk, wv], axis=1)
            ).astype(ml_dtypes.bfloat16)
            wo_sh = np.ascontiguousarray(Wo[hh * FSH : (hh + 1) * FSH, :]).astype(
                ml_dtypes.bfloat16
            )
            ones2 = np.zeros((128, 128), dtype=np.float32)
            for k in range(4):
                ones2[32 * k, 0:64] = 1.0
                ones2[32 * k + 1, 64:128] = 1.0
            in_maps.append(
                {"xT": xT, "wqkv": wqkv_sh, "wo": wo_sh, "ones2": ones2}
            )
    return in_maps


def kernel(x, Wqkv, Wo):
    global _NC_CACHE
    if _NC_CACHE is None:
        _NC_CACHE = build()
    nc = _NC_CACHE
    in_maps = _prep_inputs(x, Wqkv, Wo)
    res = run_bass_kernel_spmd(nc, in_maps, list(range(2 * B))).results
    y = np.empty((B, S, D), dtype=np.float32)
    for b in range(B):
        y[b] = res[2 * b]["y"] + res[2 * b + 1]["y"]
    return y
